# revision 15
# baseline (speedup 1.0000x reference)
import sys

import numpy as np

sys.path.insert(0, "/opt/trn_rl_repo")

B, L, C = 2, 4096, 512
H, K, DH = 8, 13, 64
SCALE = DH ** -0.5
NCORES = 8
CHUNK = 1024          # queries per core
T = 116               # queries per attention tile
NT = 9                # attention tiles per core (8*116 + 96 real + 20 pad)
KV = 1056             # kv halo slots per core
R = K // 2
NEG = -30000.0

USE_BF16 = False


def _hoff(h):
    return 512 * (h // 4) + 116 * (h % 4)


def _mm_np():
    if USE_BF16:
        import ml_dtypes
        return ml_dtypes.bfloat16
    return np.float32


def _mm_bir():
    from concourse import mybir
    return mybir.dt.bfloat16 if USE_BF16 else mybir.dt.float32


def build_masks(j, rpb):
    """Per-core mask/bias table [128 kv-slot, 3 kinds, 1024 cols].

    col = 512*(h//4) + 116*(h%4) + jq  (matches psum S^T layout exactly);
    filler cols 464:512 and 976:1024 stay NEG.
    kind 0 -> tile 0, kind 1 -> interior tiles, kind 2 -> tile NT-1.
    """
    cs = j * CHUNK
    base = cs - R
    m = np.full((128, 3, 1024), NEG, np.float32)
    for kind, t in ((0, 0), (1, 1), (2, NT - 1)):
        for jq in range(T):
            ql = t * T + jq
            if ql >= CHUNK:
                # pad query: 13 zeros -> finite denom; result discarded
                for h in range(H):
                    m[jq:jq + K, kind, _hoff(h) + jq] = 0.0
                continue
            qg = cs + ql
            sg = min(max(qg - R, 0), L - K)
            s0 = sg - base - t * T
            rel0 = sg - qg + (K - 1)
            for h in range(H):
                m[s0:s0 + K, kind, _hoff(h) + jq] = rpb[h, rel0:rel0 + K]
    return m


def _pack_x(x2d):
    # [N, 512] -> [128, 4, N]  with [ci, pi, tok] = x2d[tok, 128*pi+ci]
    n = x2d.shape[0]
    return np.ascontiguousarray(x2d.reshape(n, 4, 128).transpose(2, 1, 0))


def _halo(x, b, base):
    out = np.zeros((KV, C), np.float32)
    lo, hi = max(base, 0), min(base + KV, L)
    out[lo - base:hi - base] = x[b, lo:hi]
    return out


def build_nc(debug=False, stage=0):
    # stage: 0=full, 1=proj+Y(from qhT), 2=+head/exp, 3=+denom/recip, 4=+PV
    from concourse import bacc, mybir
    import concourse.tile as tile

    f32 = mybir.dt.float32
    mmd = _mm_bir()
    nc = bacc.Bacc("TRN2", target_bir_lowering=False, debug=debug)

    xq_d = nc.dram_tensor("xq", [128, 4, CHUNK], mmd, kind="ExternalInput")
    xk_d = nc.dram_tensor("xk", [128, 4, KV], mmd, kind="ExternalInput")
    xv_d = nc.dram_tensor("xv", [128, 4, KV], mmd, kind="ExternalInput")
    wq_d = nc.dram_tensor("wq", [128, 4, 4, 128], mmd, kind="ExternalInput")
    wk_d = nc.dram_tensor("wk", [128, 4, 4, 128], mmd, kind="ExternalInput")
    wv_d = nc.dram_tensor("wv", [128, 4, C], mmd, kind="ExternalInput")
    wo_d = nc.dram_tensor("wo", [128, 4, C], mmd, kind="ExternalInput")
    mk_d = nc.dram_tensor("mk", [128, 3, 1024], f32, kind="ExternalInput")
    on_d = nc.dram_tensor("ones", [128, 64], mmd, kind="ExternalInput")
    id_d = nc.dram_tensor("ident", [128, 128], f32, kind="ExternalInput")
    y_d = nc.dram_tensor("y", [8, 128, C], f32, kind="ExternalOutput")

    Exp = mybir.ActivationFunctionType.Exp

    with tile.TileContext(nc) as tc:
        with tc.tile_pool(name="persist", bufs=1) as pp, \
             tc.tile_pool(name="pt", bufs=2) as ptp, \
             tc.tile_pool(name="rc", bufs=2) as rcp, \
             tc.tile_pool(name="ysb", bufs=2) as yp:

            wq_s = pp.tile([128, 4, 4, 128], mmd)
            wk_s = pp.tile([128, 4, 4, 128], mmd)
            wv_s = pp.tile([128, 4, C], mmd)
            wo_s = pp.tile([128, 4, C], mmd)
            mk_s = pp.tile([128, 3, 1024], f32)
            on_s = pp.tile([128, 64], mmd)
            id_s = pp.tile([128, 128], f32)
            # qhT split: PE matmul operands must sit at partition base 0 on
            # this HW, so odd heads get their own copy with rows 0:64 zeroed
            # (even copy has rows 64:128 zeroed) and QK uses full-128
            # contraction against packed khT.
            qhT_ev = pp.tile([128, 4, KV], mmd)
            qhT_od = pp.tile([128, 4, KV], mmd)
            khT = pp.tile([128, 4, KV], mmd)
            vh = pp.tile([128, NT, C], mmd)
            oT = pp.tile([128, 4, KV], mmd)
            warm_a = pp.tile([128, 1], f32)
            warm_b = pp.tile([128, 1], f32)

            for dst, src in ((wq_s, wq_d), (wk_s, wk_d), (wv_s, wv_d),
                             (mk_s, mk_d), (on_s, on_d), (id_s, id_d),
                             (wo_s, wo_d)):
                nc.sync.dma_start(dst[:], src[:])

            # preload exp table + zero qhT halves while DMAs fly
            nc.gpsimd.memset(warm_a[:], 0.0)
            nc.scalar.activation(warm_b[:], warm_a[:], Exp)
            nc.vector.memset(qhT_ev[:], 0.0)
            nc.vector.memset(qhT_od[:], 0.0)

            # ---- projections ----
            with tc.tile_pool(name="xin", bufs=2) as xp, \
                 tc.tile_pool(name="psA", bufs=3, space="PSUM") as psA:
                xq_s = xp.tile([128, 4, KV], mmd, tag="x")
                nc.sync.dma_start(xq_s[:, :, 0:CHUNK], xq_d[:])
                xk_s = xp.tile([128, 4, KV], mmd, tag="x")
                nc.sync.dma_start(xk_s[:], xk_d[:])
                for x_s, w_s, dsts, ntok in ((xq_s, wq_s, (qhT_ev, qhT_od),
                                              CHUNK),
                                             (xk_s, wk_s, (khT,), KV)):
                    for po in range(4):
                        for off in range(0, ntok, 512):
                            w = min(512, ntok - off)
                            ps = psA.tile([128, 512], f32, tag="psA")
                            for pi in range(4):
                                nc.tensor.matmul(
                                    ps[:, 0:w], w_s[:, pi, po, :],
                                    x_s[:, pi, off:off + w],
                                    start=(pi == 0), stop=(pi == 3))
                            if len(dsts) == 1:
                                nc.scalar.copy(dsts[0][:, po, off:off + w],
                                               ps[:, 0:w])
                            else:
                                nc.scalar.copy(
                                    dsts[0][0:64, po, off:off + w],
                                    ps[0:64, 0:w])
                                nc.vector.tensor_copy(
                                    dsts[1][64:128, po, off:off + w],
                                    ps[64:128, 0:w])
                xv_s = xp.tile([128, 4, KV], mmd, tag="x")
                nc.sync.dma_start(xv_s[:], xv_d[:])
                for t in range(NT):
                    ps = psA.tile([128, 512], f32, tag="psA")
                    for pi in range(4):
                        nc.tensor.matmul(
                            ps[:], xv_s[:, pi, T * t:T * t + 128],
                            wv_s[:, pi, :], start=(pi == 0), stop=(pi == 3))
                    nc.vector.tensor_copy(vh[:, t, :], ps[:])

            # ---- neighborhood attention, S^T formulation ----
            with tc.tile_pool(name="psS", bufs=2, space="PSUM") as psS, \
                 tc.tile_pool(name="psO", bufs=2, space="PSUM") as psO, \
                 tc.tile_pool(name="psD", bufs=2, space="PSUM") as psD:

                s_tiles = {}

                def head(t):
                    if stage == 1:
                        return
                    kind = 0 if t == 0 else (2 if t == NT - 1 else 1)
                    s = psS.tile([128, 1024], f32, tag="s")
                    s_tiles[t] = s
                    nc.tensor.matmul(s[:, 0:512], id_s[:],
                                     mk_s[:, kind, 0:512],
                                     start=True, stop=False,
                                     skip_group_check=True)
                    nc.tensor.matmul(s[:, 512:1024], id_s[:],
                                     mk_s[:, kind, 512:1024],
                                     start=True, stop=False,
                                     skip_group_check=True)
                    for h in range(H):
                        po, r2 = h // 2, h % 2
                        q_src = qhT_ev if r2 == 0 else qhT_od
                        nc.tensor.matmul(
                            s[:, _hoff(h):_hoff(h) + T],
                            khT[:, po, T * t:T * t + 128],
                            q_src[:, po, T * t:T * t + T],
                            start=False, stop=True, skip_group_check=True)

                def tail(t):
                    if stage in (1, 5):
                        return
                    s = s_tiles.pop(t)
                    p = ptp.tile([128, 976], _mm_bir(), tag="p")
                    nc.scalar.activation(p[:], s[:, 0:976], Exp)
                    if stage == 2:
                        return
                    d = psD.tile([128, 4, T], f32, tag="d")
                    for h in range(H):
                        u, r2 = h // 2, h % 2
                        nc.tensor.matmul(
                            d[64 * r2:64 * r2 + 64, u, :], on_s[:],
                            p[:, _hoff(h):_hoff(h) + T],
                            start=True, stop=True, skip_group_check=True)
                    rc = rcp.tile([128, 4, T], f32, tag="rc")
                    nc.vector.reciprocal(rc[:], d[:])
                    if stage in (3, 8):
                        return
                    o = psO.tile([128, 4, T], f32, tag="o")
                    for h in range(H):
                        u, r2 = h // 2, h % 2
                        nc.tensor.matmul(
                            o[64 * r2:64 * r2 + 64, u, :],
                            vh[:, t, 64 * h:64 * h + 64],
                            p[:, _hoff(h):_hoff(h) + T],
                            start=True, stop=True, skip_group_check=True)
                    if stage == 4:
                        return
                    nc.vector.tensor_mul(oT[:, :, T * t:T * t + T], o[:], rc[:])

                for t in range(NT):
                    head(t)
                    if t >= 1:
                        tail(t - 1)
                tail(NT - 1)

            # ---- output projection ----
            o_src = oT if stage in (0,) else qhT_ev
            with tc.tile_pool(name="psY", bufs=2, space="PSUM") as psY:
                for yt in range(8):
                    ps = psY.tile([128, C], f32, tag="y")
                    for u in range(4):
                        nc.tensor.matmul(
                            ps[:], o_src[:, u, 128 * yt:128 * yt + 128],
                            wo_s[:, u, :], start=(u == 0), stop=(u == 3))
                    ysb = yp.tile([128, C], f32, tag="ysb")
                    nc.vector.tensor_copy(ysb[:], ps[:])
                    nc.sync.dma_start(y_d[yt], ysb[:])

    nc.compile()
    return nc


def prep_inputs(q, k, v, Wq, bq, Wk, bk, Wv, bv, rpb, Wo, bo):
    """Returns (in_maps list of 8 dicts, bo_eff [C])."""
    mmd = _mm_np()
    q, k, v = (np.asarray(a, np.float32) for a in (q, k, v))
    Wq, Wk, Wv, Wo = (np.asarray(a, np.float32) for a in (Wq, Wk, Wv, Wo))
    bq, bk, bv, bo = (np.asarray(a, np.float32) for a in (bq, bk, bv, bo))
    rpb = np.asarray(rpb, np.float32)
    assert not np.any(bq) and not np.any(bk), "nonzero q/k bias unsupported"

    wq_h = np.ascontiguousarray(
        (Wq * SCALE).reshape(4, 128, 4, 128).transpose(1, 0, 2, 3)).astype(mmd)
    wk_h = np.ascontiguousarray(
        Wk.reshape(4, 128, 4, 128).transpose(1, 0, 2, 3)).astype(mmd)
    wv_h = np.ascontiguousarray(Wv.reshape(4, 128, C).transpose(1, 0, 2)).astype(mmd)
    wo_h = np.ascontiguousarray(Wo.reshape(4, 128, C).transpose(1, 0, 2)).astype(mmd)
    on_h = np.ones((128, 64), mmd)
    id_h = np.eye(128, dtype=np.float32)
    masks = [build_masks(j, rpb) for j in range(4)]

    in_maps = []
    for c in range(NCORES):
        b, j = divmod(c, 4)
        cs = j * CHUNK
        base = cs - R
        in_maps.append({
            "xq": _pack_x(q[b, cs:cs + CHUNK]).astype(mmd),
            "xk": _pack_x(_halo(k, b, base)).astype(mmd),
            "xv": _pack_x(_halo(v, b, base)).astype(mmd),
            "wq": wq_h, "wk": wk_h, "wv": wv_h, "wo": wo_h,
            "mk": masks[j], "ones": on_h, "ident": id_h,
        })
    bo_eff = (bv @ Wo + bo).astype(np.float32)
    return in_maps, bo_eff


_NC_CACHE = {}


def kernel(**inputs):
    from concourse.bass_utils import run_bass_kernel_spmd

    in_maps, bo_eff = prep_inputs(**inputs)
    key = ("hw", USE_BF16)
    if key not in _NC_CACHE:
        _NC_CACHE[key] = build_nc(debug=False)
    nc = _NC_CACHE[key]

    res = run_bass_kernel_spmd(nc, in_maps, core_ids=list(range(NCORES)))
    out = np.empty((B, L, C), np.float32)
    for c in range(NCORES):
        b, j = divmod(c, 4)
        cs = j * CHUNK
        y = np.asarray(res.results[c]["y"], np.float32).reshape(CHUNK, C)
        out[b, cs:cs + CHUNK] = y + bo_eff[None, :]
    return out


# revision 16
# speedup vs baseline: 1.9679x; 1.9679x over previous
import sys

import numpy as np

sys.path.insert(0, "/opt/trn_rl_repo")

B, L, C = 2, 4096, 512
H, K, DH = 8, 13, 64
SCALE = DH ** -0.5
NCORES = 8
CHUNK = 1024          # queries per core
T = 116               # queries per attention tile
NT = 9                # attention tiles per core (8*116 + 96 real + 20 pad)
KV = 1056             # kv halo slots per core
R = K // 2
NEG = -30000.0

USE_BF16 = True


def _hoff(h):
    return 512 * (h // 4) + 116 * (h % 4)


def _mm_np():
    if USE_BF16:
        import ml_dtypes
        return ml_dtypes.bfloat16
    return np.float32


def _mm_bir():
    from concourse import mybir
    return mybir.dt.bfloat16 if USE_BF16 else mybir.dt.float32


def build_masks(j, rpb):
    """Per-core mask/bias table [128 kv-slot, 3 kinds, 1024 cols].

    col = 512*(h//4) + 116*(h%4) + jq  (matches psum S^T layout exactly);
    filler cols 464:512 and 976:1024 stay NEG.
    kind 0 -> tile 0, kind 1 -> interior tiles, kind 2 -> tile NT-1.
    """
    cs = j * CHUNK
    base = cs - R
    m = np.full((128, 3, 1024), NEG, np.float32)
    for kind, t in ((0, 0), (1, 1), (2, NT - 1)):
        for jq in range(T):
            ql = t * T + jq
            if ql >= CHUNK:
                # pad query: 13 zeros -> finite denom; result discarded
                for h in range(H):
                    m[jq:jq + K, kind, _hoff(h) + jq] = 0.0
                continue
            qg = cs + ql
            sg = min(max(qg - R, 0), L - K)
            s0 = sg - base - t * T
            rel0 = sg - qg + (K - 1)
            for h in range(H):
                m[s0:s0 + K, kind, _hoff(h) + jq] = rpb[h, rel0:rel0 + K]
    return m


def _pack_x(x2d):
    # [N, 512] -> [128, 4, N]  with [ci, pi, tok] = x2d[tok, 128*pi+ci]
    n = x2d.shape[0]
    return np.ascontiguousarray(x2d.reshape(n, 4, 128).transpose(2, 1, 0))


def _halo(x, b, base):
    out = np.zeros((KV, C), np.float32)
    lo, hi = max(base, 0), min(base + KV, L)
    out[lo - base:hi - base] = x[b, lo:hi]
    return out


def build_nc(debug=False, stage=0):
    # stage: 0=full, 1=proj+Y(from qhT), 2=+head/exp, 3=+denom/recip, 4=+PV
    from concourse import bacc, mybir
    import concourse.tile as tile

    f32 = mybir.dt.float32
    mmd = _mm_bir()
    nc = bacc.Bacc("TRN2", target_bir_lowering=False, debug=debug)

    xq_d = nc.dram_tensor("xq", [128, 4, CHUNK], mmd, kind="ExternalInput")
    xk_d = nc.dram_tensor("xk", [128, 4, KV], mmd, kind="ExternalInput")
    xv_d = nc.dram_tensor("xv", [128, 4, KV], mmd, kind="ExternalInput")
    wq_d = nc.dram_tensor("wq", [128, 4, 4, 128], mmd, kind="ExternalInput")
    wk_d = nc.dram_tensor("wk", [128, 4, 4, 128], mmd, kind="ExternalInput")
    wv_d = nc.dram_tensor("wv", [128, 4, C], mmd, kind="ExternalInput")
    wo_d = nc.dram_tensor("wo", [128, 4, C], mmd, kind="ExternalInput")
    mk_d = nc.dram_tensor("mk", [128, 3, 1024], f32, kind="ExternalInput")
    on_d = nc.dram_tensor("ones", [128, 64], mmd, kind="ExternalInput")
    id_d = nc.dram_tensor("ident", [128, 128], f32, kind="ExternalInput")
    y_d = nc.dram_tensor("y", [8, 128, C], f32, kind="ExternalOutput")

    Exp = mybir.ActivationFunctionType.Exp

    with tile.TileContext(nc) as tc:
        with tc.tile_pool(name="persist", bufs=1) as pp, \
             tc.tile_pool(name="pt", bufs=2) as ptp, \
             tc.tile_pool(name="rc", bufs=2) as rcp, \
             tc.tile_pool(name="ysb", bufs=2) as yp:

            wq_s = pp.tile([128, 4, 4, 128], mmd)
            wk_s = pp.tile([128, 4, 4, 128], mmd)
            wv_s = pp.tile([128, 4, C], mmd)
            wo_s = pp.tile([128, 4, C], mmd)
            mk_s = pp.tile([128, 3, 1024], f32)
            on_s = pp.tile([128, 64], mmd)
            id_s = pp.tile([128, 128], f32)
            # qhT split: PE matmul operands must sit at partition base 0 on
            # this HW, so odd heads get their own copy with rows 0:64 zeroed
            # (even copy has rows 64:128 zeroed) and QK uses full-128
            # contraction against packed khT.
            qhT_ev = pp.tile([128, 4, KV], mmd)
            qhT_od = pp.tile([128, 4, KV], mmd)
            khT = pp.tile([128, 4, KV], mmd)
            vh = pp.tile([128, NT, C], mmd)
            oT = pp.tile([128, 4, KV], mmd)
            warm_a = pp.tile([128, 1], f32)
            warm_b = pp.tile([128, 1], f32)

            for dst, src in ((wq_s, wq_d), (wk_s, wk_d), (wv_s, wv_d),
                             (mk_s, mk_d), (on_s, on_d), (id_s, id_d),
                             (wo_s, wo_d)):
                nc.sync.dma_start(dst[:], src[:])

            # preload exp table + zero qhT halves while DMAs fly
            nc.gpsimd.memset(warm_a[:], 0.0)
            nc.scalar.activation(warm_b[:], warm_a[:], Exp)
            nc.vector.memset(qhT_ev[:], 0.0)
            nc.vector.memset(qhT_od[:], 0.0)

            # ---- projections ----
            with tc.tile_pool(name="xin", bufs=2) as xp, \
                 tc.tile_pool(name="psA", bufs=3, space="PSUM") as psA:
                xq_s = xp.tile([128, 4, KV], mmd, tag="x")
                nc.sync.dma_start(xq_s[:, :, 0:CHUNK], xq_d[:])
                xk_s = xp.tile([128, 4, KV], mmd, tag="x")
                nc.sync.dma_start(xk_s[:], xk_d[:])
                for x_s, w_s, dsts, ntok in ((xq_s, wq_s, (qhT_ev, qhT_od),
                                              CHUNK),
                                             (xk_s, wk_s, (khT,), KV)):
                    for po in range(4):
                        for off in range(0, ntok, 512):
                            w = min(512, ntok - off)
                            ps = psA.tile([128, 512], f32, tag="psA")
                            for pi in range(4):
                                nc.tensor.matmul(
                                    ps[:, 0:w], w_s[:, pi, po, :],
                                    x_s[:, pi, off:off + w],
                                    start=(pi == 0), stop=(pi == 3))
                            if len(dsts) == 1:
                                nc.scalar.copy(dsts[0][:, po, off:off + w],
                                               ps[:, 0:w])
                            else:
                                nc.scalar.copy(
                                    dsts[0][0:64, po, off:off + w],
                                    ps[0:64, 0:w])
                                nc.vector.tensor_copy(
                                    dsts[1][64:128, po, off:off + w],
                                    ps[64:128, 0:w])
                xv_s = xp.tile([128, 4, KV], mmd, tag="x")
                nc.sync.dma_start(xv_s[:], xv_d[:])
                for t in range(NT):
                    ps = psA.tile([128, 512], f32, tag="psA")
                    for pi in range(4):
                        nc.tensor.matmul(
                            ps[:], xv_s[:, pi, T * t:T * t + 128],
                            wv_s[:, pi, :], start=(pi == 0), stop=(pi == 3))
                    nc.vector.tensor_copy(vh[:, t, :], ps[:])

            # ---- neighborhood attention, S^T formulation ----
            with tc.tile_pool(name="psS", bufs=2, space="PSUM") as psS, \
                 tc.tile_pool(name="psO", bufs=2, space="PSUM") as psO, \
                 tc.tile_pool(name="psD", bufs=2, space="PSUM") as psD:

                s_tiles = {}

                def head(t):
                    if stage == 1:
                        return
                    kind = 0 if t == 0 else (2 if t == NT - 1 else 1)
                    s = psS.tile([128, 1024], f32, tag="s")
                    s_tiles[t] = s
                    nc.tensor.matmul(s[:, 0:512], id_s[:],
                                     mk_s[:, kind, 0:512],
                                     start=True, stop=False,
                                     skip_group_check=True)
                    nc.tensor.matmul(s[:, 512:1024], id_s[:],
                                     mk_s[:, kind, 512:1024],
                                     start=True, stop=False,
                                     skip_group_check=True)
                    for h in range(H):
                        po, r2 = h // 2, h % 2
                        q_src = qhT_ev if r2 == 0 else qhT_od
                        nc.tensor.matmul(
                            s[:, _hoff(h):_hoff(h) + T],
                            khT[:, po, T * t:T * t + 128],
                            q_src[:, po, T * t:T * t + T],
                            start=False, stop=True, skip_group_check=True)

                def tail(t):
                    if stage in (1, 5):
                        return
                    s = s_tiles.pop(t)
                    p = ptp.tile([128, 976], _mm_bir(), tag="p")
                    nc.scalar.activation(p[:], s[:, 0:976], Exp)
                    if stage == 2:
                        return
                    d = psD.tile([128, 4, T], f32, tag="d")
                    for h in range(H):
                        u, r2 = h // 2, h % 2
                        nc.tensor.matmul(
                            d[64 * r2:64 * r2 + 64, u, :], on_s[:],
                            p[:, _hoff(h):_hoff(h) + T],
                            start=True, stop=True, skip_group_check=True)
                    rc = rcp.tile([128, 4, T], f32, tag="rc")
                    nc.vector.reciprocal(rc[:], d[:])
                    if stage in (3, 8):
                        return
                    o = psO.tile([128, 4, T], f32, tag="o")
                    for h in range(H):
                        u, r2 = h // 2, h % 2
                        nc.tensor.matmul(
                            o[64 * r2:64 * r2 + 64, u, :],
                            vh[:, t, 64 * h:64 * h + 64],
                            p[:, _hoff(h):_hoff(h) + T],
                            start=True, stop=True, skip_group_check=True)
                    if stage == 4:
                        return
                    nc.vector.tensor_mul(oT[:, :, T * t:T * t + T], o[:], rc[:])

                for t in range(NT):
                    head(t)
                    if t >= 1:
                        tail(t - 1)
                tail(NT - 1)

            # ---- output projection ----
            o_src = oT if stage in (0,) else qhT_ev
            with tc.tile_pool(name="psY", bufs=2, space="PSUM") as psY:
                for yt in range(8):
                    ps = psY.tile([128, C], f32, tag="y")
                    for u in range(4):
                        nc.tensor.matmul(
                            ps[:], o_src[:, u, 128 * yt:128 * yt + 128],
                            wo_s[:, u, :], start=(u == 0), stop=(u == 3))
                    ysb = yp.tile([128, C], f32, tag="ysb")
                    nc.vector.tensor_copy(ysb[:], ps[:])
                    nc.sync.dma_start(y_d[yt], ysb[:])

    nc.compile()
    return nc


def prep_inputs(q, k, v, Wq, bq, Wk, bk, Wv, bv, rpb, Wo, bo):
    """Returns (in_maps list of 8 dicts, bo_eff [C])."""
    mmd = _mm_np()
    q, k, v = (np.asarray(a, np.float32) for a in (q, k, v))
    Wq, Wk, Wv, Wo = (np.asarray(a, np.float32) for a in (Wq, Wk, Wv, Wo))
    bq, bk, bv, bo = (np.asarray(a, np.float32) for a in (bq, bk, bv, bo))
    rpb = np.asarray(rpb, np.float32)
    assert not np.any(bq) and not np.any(bk), "nonzero q/k bias unsupported"

    wq_h = np.ascontiguousarray(
        (Wq * SCALE).reshape(4, 128, 4, 128).transpose(1, 0, 2, 3)).astype(mmd)
    wk_h = np.ascontiguousarray(
        Wk.reshape(4, 128, 4, 128).transpose(1, 0, 2, 3)).astype(mmd)
    wv_h = np.ascontiguousarray(Wv.reshape(4, 128, C).transpose(1, 0, 2)).astype(mmd)
    wo_h = np.ascontiguousarray(Wo.reshape(4, 128, C).transpose(1, 0, 2)).astype(mmd)
    on_h = np.ones((128, 64), mmd)
    id_h = np.eye(128, dtype=np.float32)
    masks = [build_masks(j, rpb) for j in range(4)]

    in_maps = []
    for c in range(NCORES):
        b, j = divmod(c, 4)
        cs = j * CHUNK
        base = cs - R
        in_maps.append({
            "xq": _pack_x(q[b, cs:cs + CHUNK]).astype(mmd),
            "xk": _pack_x(_halo(k, b, base)).astype(mmd),
            "xv": _pack_x(_halo(v, b, base)).astype(mmd),
            "wq": wq_h, "wk": wk_h, "wv": wv_h, "wo": wo_h,
            "mk": masks[j], "ones": on_h, "ident": id_h,
        })
    bo_eff = (bv @ Wo + bo).astype(np.float32)
    return in_maps, bo_eff


_NC_CACHE = {}


def kernel(**inputs):
    from concourse.bass_utils import run_bass_kernel_spmd

    in_maps, bo_eff = prep_inputs(**inputs)
    key = ("hw", USE_BF16)
    if key not in _NC_CACHE:
        _NC_CACHE[key] = build_nc(debug=False)
    nc = _NC_CACHE[key]

    res = run_bass_kernel_spmd(nc, in_maps, core_ids=list(range(NCORES)))
    out = np.empty((B, L, C), np.float32)
    for c in range(NCORES):
        b, j = divmod(c, 4)
        cs = j * CHUNK
        y = np.asarray(res.results[c]["y"], np.float32).reshape(CHUNK, C)
        out[b, cs:cs + CHUNK] = y + bo_eff[None, :]
    return out


# revision 27
# speedup vs baseline: 2.2144x; 1.1253x over previous
import sys

import numpy as np

sys.path.insert(0, "/opt/trn_rl_repo")

B, L, C = 2, 4096, 512
H, K, DH = 8, 13, 64
SCALE = DH ** -0.5
NCORES = 8
CHUNK = 1024          # queries per core
T = 116               # queries per attention tile
NT = 9                # attention tiles per core (8*116 + 96 real + 20 pad)
KV = 1056             # kv halo slots per core
R = K // 2
NEG = -30000.0

USE_BF16 = True


def _hoff(h):
    return 512 * (h // 4) + 116 * (h % 4)


def _mm_np():
    if USE_BF16:
        import ml_dtypes
        return ml_dtypes.bfloat16
    return np.float32


def _mm_bir():
    from concourse import mybir
    return mybir.dt.bfloat16 if USE_BF16 else mybir.dt.float32


def build_masks(j, rpb):
    """Per-core mask/bias table [128 kv-slot, 3 kinds, 1024 cols].

    col = 512*(h//4) + 116*(h%4) + jq  (matches psum S^T layout exactly);
    filler cols 464:512 and 976:1024 stay NEG.
    kind 0 -> tile 0, kind 1 -> interior tiles, kind 2 -> tile NT-1.
    """
    cs = j * CHUNK
    base = cs - R
    m = np.full((128, 3, 1024), NEG, np.float32)
    for kind, t in ((0, 0), (1, 1), (2, NT - 1)):
        for jq in range(T):
            ql = t * T + jq
            if ql >= CHUNK:
                # pad query: 13 zeros -> finite denom; result discarded
                for h in range(H):
                    m[jq:jq + K, kind, _hoff(h) + jq] = 0.0
                continue
            qg = cs + ql
            sg = min(max(qg - R, 0), L - K)
            s0 = sg - base - t * T
            rel0 = sg - qg + (K - 1)
            for h in range(H):
                m[s0:s0 + K, kind, _hoff(h) + jq] = rpb[h, rel0:rel0 + K]
    return m


def _pack_x(x2d):
    # [N, 512] -> [128, 4, N]  with [ci, pi, tok] = x2d[tok, 128*pi+ci]
    n = x2d.shape[0]
    return np.ascontiguousarray(x2d.reshape(n, 4, 128).transpose(2, 1, 0))


def _halo(x, b, base):
    out = np.zeros((KV, C), np.float32)
    lo, hi = max(base, 0), min(base + KV, L)
    out[lo - base:hi - base] = x[b, lo:hi]
    return out


def build_nc(debug=False, stage=0):
    # stage: 0=full, 1=proj+Y(from qhT), 2=+head/exp, 3=+denom/recip, 4=+PV
    from concourse import bacc, mybir
    import concourse.tile as tile

    f32 = mybir.dt.float32
    mmd = _mm_bir()
    nc = bacc.Bacc("TRN2", target_bir_lowering=False, debug=debug)

    xq_d = nc.dram_tensor("xq", [128, 4, CHUNK], mmd, kind="ExternalInput")
    xk_d = nc.dram_tensor("xk", [128, 4, KV], mmd, kind="ExternalInput")
    xv_d = nc.dram_tensor("xv", [128, 4, KV], mmd, kind="ExternalInput")
    wq_d = nc.dram_tensor("wq", [128, 4, 4, 128], mmd, kind="ExternalInput")
    wk_d = nc.dram_tensor("wk", [128, 4, 4, 128], mmd, kind="ExternalInput")
    wv_d = nc.dram_tensor("wv", [128, 4, C], mmd, kind="ExternalInput")
    wo_d = nc.dram_tensor("wo", [128, 4, C], mmd, kind="ExternalInput")
    mk_d = nc.dram_tensor("mk", [128, 3, 2, 512], f32, kind="ExternalInput")
    on_d = nc.dram_tensor("ones", [128, 64], mmd, kind="ExternalInput")
    y_d = nc.dram_tensor("y", [8, 128, C], f32, kind="ExternalOutput")

    Exp = mybir.ActivationFunctionType.Exp

    with tile.TileContext(nc) as tc:
        with tc.tile_pool(name="persist", bufs=1) as pp, \
             tc.tile_pool(name="pt", bufs=2) as ptp, \
             tc.tile_pool(name="rc", bufs=2) as rcp, \
             tc.tile_pool(name="ysb", bufs=2) as yp:

            wq_s = pp.tile([128, 4, 4, 128], mmd)
            wk_s = pp.tile([128, 4, 4, 128], mmd)
            wv_s = pp.tile([128, 4, C], mmd)
            wo_s = pp.tile([128, 4, C], mmd)
            mk_s = pp.tile([128, 3, 2, 512], f32)
            on_s = pp.tile([128, 64], mmd)
            # qhT split: PE matmul operands must sit at partition base 0 on
            # this HW, so odd heads get their own copy with rows 0:64 zeroed
            # (even copy has rows 64:128 zeroed) and QK uses full-128
            # contraction against packed khT.
            qhT_ev = pp.tile([128, 4, KV], mmd)
            qhT_od = pp.tile([128, 4, KV], mmd)
            khT = pp.tile([128, 4, KV], mmd)
            vh = pp.tile([128, NT, C], mmd)
            oT = pp.tile([128, 4, KV], mmd)
            warm_a = pp.tile([128, 1], f32)
            warm_b = pp.tile([128, 1], f32)

            for dst, src in ((wq_s, wq_d), (wk_s, wk_d), (wv_s, wv_d),
                             (mk_s, mk_d), (on_s, on_d), (wo_s, wo_d)):
                nc.sync.dma_start(dst[:], src[:])

            # preload exp table + zero qhT halves while DMAs fly
            nc.gpsimd.memset(warm_a[:], 0.0)
            nc.scalar.activation(warm_b[:], warm_a[:], Exp)
            nc.vector.memset(qhT_ev[:], 0.0)
            nc.vector.memset(qhT_od[:], 0.0)

            # ---- projections ----
            with tc.tile_pool(name="xin", bufs=2) as xp, \
                 tc.tile_pool(name="psA", bufs=3, space="PSUM") as psA:
                xq_s = xp.tile([128, 4, KV], mmd, tag="x")
                nc.sync.dma_start(xq_s[:, :, 0:CHUNK], xq_d[:])
                xk_s = xp.tile([128, 4, KV], mmd, tag="x")
                nc.sync.dma_start(xk_s[:], xk_d[:])
                for x_s, w_s, dsts, ntok in ((xq_s, wq_s, (qhT_ev, qhT_od),
                                              CHUNK),
                                             (xk_s, wk_s, (khT,), KV)):
                    for po in range(4):
                        for off in range(0, ntok, 512):
                            w = min(512, ntok - off)
                            ps = psA.tile([128, 512], f32, tag="psA")
                            for pi in range(4):
                                nc.tensor.matmul(
                                    ps[:, 0:w], w_s[:, pi, po, :],
                                    x_s[:, pi, off:off + w],
                                    start=(pi == 0), stop=(pi == 3))
                            if len(dsts) == 1:
                                nc.scalar.copy(dsts[0][:, po, off:off + w],
                                               ps[:, 0:w])
                            else:
                                nc.scalar.copy(
                                    dsts[0][0:64, po, off:off + w],
                                    ps[0:64, 0:w])
                                nc.vector.tensor_copy(
                                    dsts[1][64:128, po, off:off + w],
                                    ps[64:128, 0:w])
                xv_s = xp.tile([128, 4, KV], mmd, tag="x")
                nc.sync.dma_start(xv_s[:], xv_d[:])
                for t in range(NT):
                    ps = psA.tile([128, 512], f32, tag="psA")
                    for pi in range(4):
                        nc.tensor.matmul(
                            ps[:], xv_s[:, pi, T * t:T * t + 128],
                            wv_s[:, pi, :], start=(pi == 0), stop=(pi == 3))
                    nc.vector.tensor_copy(vh[:, t, :], ps[:])

            # ---- neighborhood attention, S^T formulation ----
            with tc.tile_pool(name="psS", bufs=2, space="PSUM") as psS, \
                 tc.tile_pool(name="psO", bufs=2, space="PSUM") as psO, \
                 tc.tile_pool(name="psD", bufs=2, space="PSUM") as psD:

                s_tiles = {}

                def head(t):
                    if stage == 1:
                        return
                    s = psS.tile([128, 2, 512], f32, tag="s")
                    s_tiles[t] = s
                    for h in range(H):
                        po, r2 = h // 2, h % 2
                        q_src = qhT_ev if r2 == 0 else qhT_od
                        hc = T * (h % 4)
                        nc.tensor.matmul(
                            s[:, h // 4, hc:hc + T],
                            khT[:, po, T * t:T * t + 128],
                            q_src[:, po, T * t:T * t + T],
                            start=True, stop=True, skip_group_check=True)

                def tail(t):
                    if stage in (1, 5):
                        return
                    kind = 0 if t == 0 else (2 if t == NT - 1 else 1)
                    s = s_tiles.pop(t)
                    nc.vector.tensor_add(s[:, :, 0:4 * T], s[:, :, 0:4 * T],
                                         mk_s[:, kind, :, 0:4 * T])
                    p = ptp.tile([128, 2, 4 * T], _mm_bir(), tag="p")
                    nc.scalar.activation(p[:], s[:, :, 0:4 * T], Exp)
                    if stage == 2:
                        return
                    d = psD.tile([128, 4, T], f32, tag="d")
                    for h in range(H):
                        u, r2 = h // 2, h % 2
                        hc = T * (h % 4)
                        nc.tensor.matmul(
                            d[64 * r2:64 * r2 + 64, u, :], on_s[:],
                            p[:, h // 4, hc:hc + T],
                            start=True, stop=True, skip_group_check=True)
                    rc = rcp.tile([128, 4, T], f32, tag="rc")
                    nc.vector.reciprocal_approx_fast(rc[:], d[:])
                    if stage in (3, 8):
                        return
                    o = psO.tile([128, 4, T], f32, tag="o")
                    for h in range(H):
                        u, r2 = h // 2, h % 2
                        hc = T * (h % 4)
                        nc.tensor.matmul(
                            o[64 * r2:64 * r2 + 64, u, :],
                            vh[:, t, 64 * h:64 * h + 64],
                            p[:, h // 4, hc:hc + T],
                            start=True, stop=True, skip_group_check=True)
                    if stage == 4:
                        return
                    nc.vector.tensor_mul(oT[:, :, T * t:T * t + T], o[:], rc[:])

                for t in range(NT):
                    head(t)
                    if t >= 1:
                        tail(t - 1)
                tail(NT - 1)

            # ---- output projection ----
            o_src = oT if stage in (0,) else qhT_ev
            with tc.tile_pool(name="psY", bufs=2, space="PSUM") as psY:
                for yt in range(8):
                    ps = psY.tile([128, C], f32, tag="y")
                    for u in range(4):
                        nc.tensor.matmul(
                            ps[:], o_src[:, u, 128 * yt:128 * yt + 128],
                            wo_s[:, u, :], start=(u == 0), stop=(u == 3))
                    ysb = yp.tile([128, C], f32, tag="ysb")
                    nc.vector.tensor_copy(ysb[:], ps[:])
                    nc.sync.dma_start(y_d[yt], ysb[:])

    nc.compile()
    return nc


def prep_inputs(q, k, v, Wq, bq, Wk, bk, Wv, bv, rpb, Wo, bo):
    """Returns (in_maps list of 8 dicts, bo_eff [C])."""
    mmd = _mm_np()
    q, k, v = (np.asarray(a, np.float32) for a in (q, k, v))
    Wq, Wk, Wv, Wo = (np.asarray(a, np.float32) for a in (Wq, Wk, Wv, Wo))
    bq, bk, bv, bo = (np.asarray(a, np.float32) for a in (bq, bk, bv, bo))
    rpb = np.asarray(rpb, np.float32)
    assert not np.any(bq) and not np.any(bk), "nonzero q/k bias unsupported"

    wq_h = np.ascontiguousarray(
        (Wq * SCALE).reshape(4, 128, 4, 128).transpose(1, 0, 2, 3)).astype(mmd)
    wk_h = np.ascontiguousarray(
        Wk.reshape(4, 128, 4, 128).transpose(1, 0, 2, 3)).astype(mmd)
    wv_h = np.ascontiguousarray(Wv.reshape(4, 128, C).transpose(1, 0, 2)).astype(mmd)
    wo_h = np.ascontiguousarray(Wo.reshape(4, 128, C).transpose(1, 0, 2)).astype(mmd)
    on_h = np.ones((128, 64), mmd)
    masks = [build_masks(j, rpb) for j in range(4)]

    in_maps = []
    for c in range(NCORES):
        b, j = divmod(c, 4)
        cs = j * CHUNK
        base = cs - R
        in_maps.append({
            "xq": _pack_x(q[b, cs:cs + CHUNK]).astype(mmd),
            "xk": _pack_x(_halo(k, b, base)).astype(mmd),
            "xv": _pack_x(_halo(v, b, base)).astype(mmd),
            "wq": wq_h, "wk": wk_h, "wv": wv_h, "wo": wo_h,
            "mk": masks[j].reshape(128, 3, 2, 512), "ones": on_h,
        })
    bo_eff = (bv @ Wo + bo).astype(np.float32)
    return in_maps, bo_eff


_NC_CACHE = {}


def kernel(**inputs):
    from concourse.bass_utils import run_bass_kernel_spmd

    in_maps, bo_eff = prep_inputs(**inputs)
    key = ("hw", USE_BF16)
    if key not in _NC_CACHE:
        _NC_CACHE[key] = build_nc(debug=False)
    nc = _NC_CACHE[key]

    res = run_bass_kernel_spmd(nc, in_maps, core_ids=list(range(NCORES)))
    out = np.empty((B, L, C), np.float32)
    for c in range(NCORES):
        b, j = divmod(c, 4)
        cs = j * CHUNK
        y = np.asarray(res.results[c]["y"], np.float32).reshape(CHUNK, C)
        out[b, cs:cs + CHUNK] = y + bo_eff[None, :]
    return out


# revision 32
# speedup vs baseline: 2.4583x; 1.1101x over previous
import sys

import numpy as np

sys.path.insert(0, "/opt/trn_rl_repo")

B, L, C = 2, 4096, 512
H, K, DH = 8, 13, 64
SCALE = DH ** -0.5
NCORES = 8
CHUNK = 1024          # queries per core
T = 116               # queries per attention tile
NT = 9                # attention tiles per core (8*116 + 96 real + 20 pad)
KV = 1056             # kv halo slots per core
R = K // 2
NEG = -30000.0

USE_BF16 = True


def _hoff(h):
    return 512 * (h // 4) + 116 * (h % 4)


def _mm_np():
    if USE_BF16:
        import ml_dtypes
        return ml_dtypes.bfloat16
    return np.float32


def _mm_bir():
    from concourse import mybir
    return mybir.dt.bfloat16 if USE_BF16 else mybir.dt.float32


def build_masks(j, rpb):
    """Per-core mask/bias table [128 kv-slot, 3 kinds, 1024 cols].

    col = 512*(h//4) + 116*(h%4) + jq  (matches psum S^T layout exactly);
    filler cols 464:512 and 976:1024 stay NEG.
    kind 0 -> tile 0, kind 1 -> interior tiles, kind 2 -> tile NT-1.
    """
    cs = j * CHUNK
    base = cs - R
    m = np.full((128, 3, 1024), NEG, np.float32)
    for kind, t in ((0, 0), (1, 1), (2, NT - 1)):
        for jq in range(T):
            ql = t * T + jq
            if ql >= CHUNK:
                # pad query: 13 zeros -> finite denom; result discarded
                for h in range(H):
                    m[jq:jq + K, kind, _hoff(h) + jq] = 0.0
                continue
            qg = cs + ql
            sg = min(max(qg - R, 0), L - K)
            s0 = sg - base - t * T
            rel0 = sg - qg + (K - 1)
            for h in range(H):
                m[s0:s0 + K, kind, _hoff(h) + jq] = rpb[h, rel0:rel0 + K]
    return m


def _pack_x(x2d):
    # [N, 512] -> [128, 4, N]  with [ci, pi, tok] = x2d[tok, 128*pi+ci]
    n = x2d.shape[0]
    return np.ascontiguousarray(x2d.reshape(n, 4, 128).transpose(2, 1, 0))


def _halo(x, b, base):
    out = np.zeros((KV, C), np.float32)
    lo, hi = max(base, 0), min(base + KV, L)
    out[lo - base:hi - base] = x[b, lo:hi]
    return out


def build_nc(debug=False, stage=0):
    # stage: 0=full, 1=proj+Y(from qhT), 2=+head/exp, 3=+denom/recip, 4=+PV
    from concourse import bacc, mybir
    import concourse.tile as tile

    f32 = mybir.dt.float32
    mmd = _mm_bir()
    nc = bacc.Bacc("TRN2", target_bir_lowering=False, debug=debug)

    xq_d = nc.dram_tensor("xq", [128, 4, CHUNK], mmd, kind="ExternalInput")
    xk_d = nc.dram_tensor("xk", [128, 4, KV], mmd, kind="ExternalInput")
    xv_d = nc.dram_tensor("xv", [128, 4, KV], mmd, kind="ExternalInput")
    wq_d = nc.dram_tensor("wq", [128, 4, 4, 128], mmd, kind="ExternalInput")
    wk_d = nc.dram_tensor("wk", [128, 4, 4, 128], mmd, kind="ExternalInput")
    wv_d = nc.dram_tensor("wv", [128, 4, C], mmd, kind="ExternalInput")
    wo_d = nc.dram_tensor("wo", [128, 4, C], mmd, kind="ExternalInput")
    mk_d = nc.dram_tensor("mk", [128, 3, 2, 512], mmd, kind="ExternalInput")
    on_d = nc.dram_tensor("ones", [128, 64], mmd, kind="ExternalInput")
    y_d = nc.dram_tensor("y", [8, 128, C], f32, kind="ExternalOutput")

    Exp = mybir.ActivationFunctionType.Exp

    with tile.TileContext(nc) as tc:
        with tc.tile_pool(name="persist", bufs=1) as pp, \
             tc.tile_pool(name="pt", bufs=2) as ptp, \
             tc.tile_pool(name="rc", bufs=2) as rcp, \
             tc.tile_pool(name="ysb", bufs=2) as yp:

            wq_s = pp.tile([128, 4, 4, 128], mmd)
            wk_s = pp.tile([128, 4, 4, 128], mmd)
            wv_s = pp.tile([128, 4, C], mmd)
            wo_s = pp.tile([128, 4, C], mmd)
            mk_s = pp.tile([128, 3, 2, 512], mmd)
            on_s = pp.tile([128, 64], mmd)
            # qhT split: PE matmul operands must sit at partition base 0 on
            # this HW, so odd heads get their own copy with rows 0:64 zeroed
            # (even copy has rows 64:128 zeroed) and QK uses full-128
            # contraction against packed khT.
            qhT_ev = pp.tile([128, 4, KV], mmd)
            qhT_od = pp.tile([128, 4, KV], mmd)
            khT = pp.tile([128, 4, KV], mmd)
            vh = pp.tile([128, NT, C], mmd)
            oT = pp.tile([128, 4, KV], mmd)
            warm_a = pp.tile([128, 1], f32)
            warm_b = pp.tile([128, 1], f32)

            # preload exp table + zero qhT halves while DMAs fly
            nc.gpsimd.memset(warm_a[:], 0.0)
            nc.scalar.activation(warm_b[:], warm_a[:], Exp)
            nc.vector.memset(qhT_ev[:], 0.0)
            nc.vector.memset(qhT_od[:], 0.0)

            # ---- projections ----
            # DMAs ordered by first use and split across two queues so the
            # Q projection starts as soon as xq+wq land.
            with tc.tile_pool(name="xin", bufs=2) as xp, \
                 tc.tile_pool(name="psA", bufs=3, space="PSUM") as psA:
                xq_s = xp.tile([128, 4, KV], mmd, tag="x")
                nc.sync.dma_start(xq_s[:, :, 0:CHUNK], xq_d[:])
                nc.sync.dma_start(wq_s[:], wq_d[:])
                xk_s = xp.tile([128, 4, KV], mmd, tag="x")
                nc.gpsimd.dma_start(xk_s[:], xk_d[:])
                nc.gpsimd.dma_start(wk_s[:], wk_d[:])
                nc.sync.dma_start(wv_s[:], wv_d[:])
                nc.gpsimd.dma_start(on_s[:], on_d[:])
                nc.sync.dma_start(mk_s[:], mk_d[:])
                nc.sync.dma_start(wo_s[:], wo_d[:])
                for x_s, w_s, dsts, ntok in ((xq_s, wq_s, (qhT_ev, qhT_od),
                                              CHUNK),
                                             (xk_s, wk_s, (khT,), KV)):
                    for po in range(4):
                        for off in range(0, ntok, 512):
                            w = min(512, ntok - off)
                            ps = psA.tile([128, 512], f32, tag="psA")
                            for pi in range(4):
                                nc.tensor.matmul(
                                    ps[:, 0:w], w_s[:, pi, po, :],
                                    x_s[:, pi, off:off + w],
                                    start=(pi == 0), stop=(pi == 3))
                            if len(dsts) == 1:
                                nc.scalar.copy(dsts[0][:, po, off:off + w],
                                               ps[:, 0:w])
                            else:
                                nc.scalar.copy(
                                    dsts[0][0:64, po, off:off + w],
                                    ps[0:64, 0:w])
                                nc.vector.tensor_copy(
                                    dsts[1][64:128, po, off:off + w],
                                    ps[64:128, 0:w])
                xv_s = xp.tile([128, 4, KV], mmd, tag="x")
                nc.gpsimd.dma_start(xv_s[:], xv_d[:])
                for t in range(NT):
                    ps = psA.tile([128, 512], f32, tag="psA")
                    for pi in range(4):
                        nc.tensor.matmul(
                            ps[:], xv_s[:, pi, T * t:T * t + 128],
                            wv_s[:, pi, :], start=(pi == 0), stop=(pi == 3))
                    nc.vector.tensor_copy(vh[:, t, :], ps[:])

            # ---- neighborhood attention, S^T formulation ----
            with tc.tile_pool(name="psS", bufs=2, space="PSUM") as psS, \
                 tc.tile_pool(name="psO", bufs=2, space="PSUM") as psO, \
                 tc.tile_pool(name="psD", bufs=2, space="PSUM") as psD:

                s_tiles = {}

                def head(t):
                    if stage == 1:
                        return
                    s = psS.tile([128, 2, 512], f32, tag="s")
                    s_tiles[t] = s
                    for h in range(H):
                        po, r2 = h // 2, h % 2
                        q_src = qhT_ev if r2 == 0 else qhT_od
                        hc = T * (h % 4)
                        nc.tensor.matmul(
                            s[:, h // 4, hc:hc + T],
                            khT[:, po, T * t:T * t + 128],
                            q_src[:, po, T * t:T * t + T],
                            start=True, stop=True, skip_group_check=True)

                def tail(t):
                    if stage in (1, 5):
                        return
                    kind = 0 if t == 0 else (2 if t == NT - 1 else 1)
                    s = s_tiles.pop(t)
                    nc.vector.tensor_add(s[:, :, 0:4 * T], s[:, :, 0:4 * T],
                                         mk_s[:, kind, :, 0:4 * T])
                    p = ptp.tile([128, 2, 4 * T], _mm_bir(), tag="p")
                    nc.scalar.activation(p[:], s[:, :, 0:4 * T], Exp)
                    if stage == 2:
                        return
                    d = psD.tile([128, 4, T], f32, tag="d")
                    for h in range(H):
                        u, r2 = h // 2, h % 2
                        hc = T * (h % 4)
                        nc.tensor.matmul(
                            d[64 * r2:64 * r2 + 64, u, :], on_s[:],
                            p[:, h // 4, hc:hc + T],
                            start=True, stop=True, skip_group_check=True)
                    rc = rcp.tile([128, 4, T], f32, tag="rc")
                    nc.vector.reciprocal_approx_fast(rc[:], d[:])
                    if stage in (3, 8):
                        return
                    o = psO.tile([128, 4, T], f32, tag="o")
                    for h in range(H):
                        u, r2 = h // 2, h % 2
                        hc = T * (h % 4)
                        nc.tensor.matmul(
                            o[64 * r2:64 * r2 + 64, u, :],
                            vh[:, t, 64 * h:64 * h + 64],
                            p[:, h // 4, hc:hc + T],
                            start=True, stop=True, skip_group_check=True)
                    if stage == 4:
                        return
                    nc.vector.tensor_mul(oT[:, :, T * t:T * t + T], o[:], rc[:])

                for t in range(NT):
                    head(t)
                    if t >= 1:
                        tail(t - 1)
                tail(NT - 1)

            # ---- output projection ----
            o_src = oT if stage in (0,) else qhT_ev
            with tc.tile_pool(name="psY", bufs=2, space="PSUM") as psY:
                for yt in range(8):
                    ps = psY.tile([128, C], f32, tag="y")
                    for u in range(4):
                        nc.tensor.matmul(
                            ps[:], o_src[:, u, 128 * yt:128 * yt + 128],
                            wo_s[:, u, :], start=(u == 0), stop=(u == 3))
                    ysb = yp.tile([128, C], f32, tag="ysb")
                    nc.vector.tensor_copy(ysb[:], ps[:])
                    nc.sync.dma_start(y_d[yt], ysb[:])

    nc.compile()
    return nc


def prep_inputs(q, k, v, Wq, bq, Wk, bk, Wv, bv, rpb, Wo, bo):
    """Returns (in_maps list of 8 dicts, bo_eff [C])."""
    mmd = _mm_np()
    q, k, v = (np.asarray(a, np.float32) for a in (q, k, v))
    Wq, Wk, Wv, Wo = (np.asarray(a, np.float32) for a in (Wq, Wk, Wv, Wo))
    bq, bk, bv, bo = (np.asarray(a, np.float32) for a in (bq, bk, bv, bo))
    rpb = np.asarray(rpb, np.float32)
    assert not np.any(bq) and not np.any(bk), "nonzero q/k bias unsupported"

    wq_h = np.ascontiguousarray(
        (Wq * SCALE).reshape(4, 128, 4, 128).transpose(1, 0, 2, 3)).astype(mmd)
    wk_h = np.ascontiguousarray(
        Wk.reshape(4, 128, 4, 128).transpose(1, 0, 2, 3)).astype(mmd)
    wv_h = np.ascontiguousarray(Wv.reshape(4, 128, C).transpose(1, 0, 2)).astype(mmd)
    wo_h = np.ascontiguousarray(Wo.reshape(4, 128, C).transpose(1, 0, 2)).astype(mmd)
    on_h = np.ones((128, 64), mmd)
    masks = [build_masks(j, rpb) for j in range(4)]

    in_maps = []
    for c in range(NCORES):
        b, j = divmod(c, 4)
        cs = j * CHUNK
        base = cs - R
        in_maps.append({
            "xq": _pack_x(q[b, cs:cs + CHUNK]).astype(mmd),
            "xk": _pack_x(_halo(k, b, base)).astype(mmd),
            "xv": _pack_x(_halo(v, b, base)).astype(mmd),
            "wq": wq_h, "wk": wk_h, "wv": wv_h, "wo": wo_h,
            "mk": masks[j].reshape(128, 3, 2, 512).astype(mmd), "ones": on_h,
        })
    bo_eff = (bv @ Wo + bo).astype(np.float32)
    return in_maps, bo_eff


_NC_CACHE = {}


def kernel(**inputs):
    from concourse.bass_utils import run_bass_kernel_spmd

    in_maps, bo_eff = prep_inputs(**inputs)
    key = ("hw", USE_BF16)
    if key not in _NC_CACHE:
        _NC_CACHE[key] = build_nc(debug=False)
    nc = _NC_CACHE[key]

    res = run_bass_kernel_spmd(nc, in_maps, core_ids=list(range(NCORES)))
    out = np.empty((B, L, C), np.float32)
    for c in range(NCORES):
        b, j = divmod(c, 4)
        cs = j * CHUNK
        y = np.asarray(res.results[c]["y"], np.float32).reshape(CHUNK, C)
        out[b, cs:cs + CHUNK] = y + bo_eff[None, :]
    return out
